# revision 12
# baseline (speedup 1.0000x reference)
"""Trainium2 Bass kernel for the 3-layer weighted GraphConv encoder (v3).

Strategy (8 NeuronCores, SPMD single NEFF), all-bf16 datapath:
- Layer 1 (movie->movie): edges sharded by DST range. Messages x_meas[src_m]
  are PRE-GATHERED ON HOST into a contiguous bf16 stream (the indices are
  static, so the per-row SWDGE descriptor toll -- ~5-8ns/row of serial Q7
  time -- is avoided entirely). Segment-sum on the tensor engine via
  S[e,s] = (dstloc[e]==s) selection matrices; feature-major agg kept in SBUF.
- Layers 2+3 (movie->user, shared edge set): edges sharded by SRC range so
  the layer-3 gather of movie_x (device-computed) reads only the core-local
  [RM, D] bf16 table via dma_gather (256B rows). Layer-2 messages
  x_meas[src_b] come from a host-pregathered bf16 stream. One shared
  S = (dstloc==s)*sigmoid(ew) per 128-edge chunk (256-dst windows) feeds two
  matmuls (acc2 from the host stream, acc3 from the gathered movie rows).
- Partial [128, N_D]-feature-major aggregates are reduced across cores with
  5 chunked bf16 ReduceScatters. Windows are processed in (group, block,
  sub-window) order so each RS piece completes early; each RS is emitted one
  group late on the Pool queue so its semaphore wait never stalls the
  gather stream. The small dense linears run replicated per-core at the end.

The per-window chunk schedule is data-dependent; it is computed from the
actual inputs at kernel() time (max over cores per window) and baked into
the program, with per-core padding to the shared schedule (padded slots
have msg rows = 0 and dstloc = -1 so they contribute nothing).
"""

import math

import ml_dtypes
import numpy as np
import orjson

import concourse.bass as bass
import concourse.mybir as mybir
import concourse.tile as tile
from concourse import library_config
from concourse.library_overlay import lower_extended_insts
from concourse.bass_utils import run_bass_kernel_spmd

BF16NP = ml_dtypes.bfloat16

# ---------------------------------------------------------------- BIR legalize
# The pinned walrus build accepts at most one sync-wait and one sync-update
# per instruction; Tile emits several. Hoist extras onto EventSemaphore nops.
_DMA_OPCODES = {
    "DMACopy", "TensorLoad", "TensorSave", "ISA", "CollectiveCompute",
    "DmaTranspose", "TriggerDma",
}
_lg_counter = [0]


def _lg_nop(inst, wait=None, update=None):
    _lg_counter[0] += 1
    return {
        "name": f"lg{_lg_counter[0]}",
        "opcode": "EventSemaphore",
        "engine": inst["engine"],
        "ins": [],
        "outs": [],
        "debug": inst.get("debug", 0),
        "sync_info": {
            "on_wait": [wait] if wait else [],
            "on_update": [update] if update else [],
        },
    }


def _lg_walk(block, stats):
    out = []
    for inst in block.get("instructions", []):
        si = inst.get("sync_info")
        trailing = []
        if si:
            ows = si.get("on_wait") or []
            if len(ows) > 1:
                stats[0] += len(ows) - 1
                for w in ows[:-1]:
                    out.append(_lg_nop(inst, wait=w))
                si["on_wait"] = [ows[-1]]
            ous = si.get("on_update") or []
            if len(ous) > 1 and inst.get("opcode") not in _DMA_OPCODES:
                stats[1] += len(ous) - 1
                for u in ous[1:]:
                    trailing.append(_lg_nop(inst, update=u))
                si["on_update"] = [ous[0]]
        out.append(inst)
        out.extend(trailing)
    block["instructions"] = out
    for sb in block.get("blocks") or []:
        _lg_walk(sb, stats)


def legalize_bir_json(bir_json: bytes) -> bytes:
    d = orjson.loads(bir_json)
    stats = [0, 0]
    for fn in d.get("functions", []):
        for b in fn.get("blocks", []):
            _lg_walk(b, stats)
    return orjson.dumps(d)


def _install_birfix():
    import concourse.bass_utils as bu
    import concourse.bass2jax as b2j

    if getattr(bu, "_birfix_installed", False):
        return
    orig = bu.compile_bir_kernel

    def wrapper(bir_json, tmpdir, neff_name="file.neff"):
        return orig(legalize_bir_json(bir_json), tmpdir, neff_name)

    bu.compile_bir_kernel = wrapper
    bu._birfix_installed = True
    b2j.compile_bir_kernel = wrapper


# ------------------------------------------------------------------- constants
N_M, N_D, E, D, H, O = 50000, 20000, 600000, 128, 128, 64
NC = 8
P = 128
WW = 128                  # layer-2/3 dst window width
RM = N_M // NC            # 6250 movie rows per core
RU = N_D // NC            # 2500 user rows per core
W1 = math.ceil(RM / P)    # 49 windows for layer 1
W2B = math.ceil(RU / WW)  # 10 user windows per dst block
NWG = 5                   # ReduceScatter groups (2 sub-windows each)
WPG = W2B // NWG          # sub-windows per group (2)
WU = NC * W2B             # 80 user windows for layers 2+3
NB = 8                    # chunks per dma_gather batch (>1024 idx/call faults)
SB1 = 16                  # layer-1 stream chunks per DMA batch
F32 = mybir.dt.float32
BF16 = mybir.dt.bfloat16
I16 = mybir.dt.int16


# ---------------------------------------------------------------- host-side prep
def _schedule(core, win, nwin):
    """Shared chunk schedule + per-edge slot positions.

    Returns (sched [nwin] = chunks per window, pos [E'] = slot index of each
    edge within its core's stream). All cores share sched; each core's stream
    is sched.sum()*P slots with window w's run at off[w]."""
    counts = np.zeros((NC, nwin), np.int64)
    np.add.at(counts, (core, win), 1)
    sched = (counts.max(axis=0) + P - 1) // P
    run_len = sched * P
    off = np.concatenate(([0], np.cumsum(run_len)[:-1]))

    # rank of each edge within its (core, win) bucket
    order = np.lexsort((win, core))
    inv = np.empty_like(order)
    inv[order] = np.arange(len(order))
    flat = core * nwin + win
    sort_flat = flat[order]
    starts = np.concatenate(([0], np.nonzero(np.diff(sort_flat))[0] + 1))
    run_start = np.zeros(len(order), np.int64)
    run_start[starts] = starts
    run_start = np.maximum.accumulate(run_start)
    rank = (np.arange(len(order)) - run_start)[inv]

    pos = off[win] + rank
    return sched, pos


def _pack_msg_stream(pos_c, rows_bf16, nslots):
    """Scatter pregathered bf16 rows [n, D] into the DMA stream layout
    [P, nchunks*D]: slot s -> (chunk s//P, partition s%P)."""
    arr = np.zeros((nslots, D), BF16NP)
    arr[pos_c] = rows_bf16
    nch = nslots // P
    return np.ascontiguousarray(
        arr.reshape(nch, P, D).transpose(1, 0, 2).reshape(P, nch * D))


def _pack_col(pos_c, vals, nslots, fill):
    arr = np.full(nslots, fill, np.float32)
    arr[pos_c] = vals
    return np.ascontiguousarray(arr.reshape(-1, P).T)


def _pack_dstew(pos_c, dst_vals, ew_vals, nslots):
    """Per dma_gather call k: cols [k*2NB, k*2NB+NB) = negated dstloc chunks,
    cols [k*2NB+NB, k*2NB+2NB) = raw edge weights."""
    dc = _pack_col(pos_c, -dst_vals, nslots, 1.0)      # [P, nch]
    ec = _pack_col(pos_c, ew_vals, nslots, 0.0)
    nch = nslots // P
    ncalls = math.ceil(nch / NB)
    out = np.zeros((P, ncalls * 2 * NB), np.float32)
    for k in range(ncalls):
        nb = min(NB, nch - k * NB)
        out[:, k * 2 * NB: k * 2 * NB + nb] = dc[:, k * NB: k * NB + nb]
        out[:, k * 2 * NB + NB: k * 2 * NB + NB + nb] = ec[:, k * NB: k * NB + nb]
    return out


def _pack_idx_dma(pos_c, idx_vals, nslots):
    """idx stream -> DMA layout [P, ncalls*NB*8]: per dma_gather call k
    (NB chunks), index j -> partition j%16 (replicated x8), col k*NB*8+j//16."""
    idx_a = np.zeros(nslots, np.int16)
    idx_a[pos_c] = idx_vals.astype(np.int16)
    nchunks = nslots // P
    ncalls = math.ceil(nchunks / NB)
    out = np.zeros((P, ncalls * NB * 8), np.int16)
    for k in range(ncalls):
        nb = min(NB, nchunks - k * NB)
        call = idx_a[k * NB * P: k * NB * P + nb * P]
        blk = call.reshape(nb * 8, 16).T               # [16, nb*8]
        out[:, k * NB * 8: k * NB * 8 + nb * 8] = np.tile(blk, (8, 1))
    return out


# --------------------------------------------------------------- device program
def _build_program(sched1, sched3):
    nc = bass.Bass(trn_type="TRN2", num_devices=NC, num_swdge_queues=4)

    nch1 = int(sched1.sum())
    nch3 = int(sched3.sum())
    ncalls3 = math.ceil(nch3 / NB)

    # ---- kernel I/O ----
    s1msg = nc.dram_tensor("s1msg", [P, nch1 * D], BF16, kind="ExternalInput")
    s1dst = nc.dram_tensor("s1dst", [P, nch1], F32, kind="ExternalInput")
    s2msg = nc.dram_tensor("s2msg", [P, nch3 * D], BF16, kind="ExternalInput")
    l3idx = nc.dram_tensor("l3idx", [P, ncalls3 * NB * 8], I16,
                           kind="ExternalInput")
    l3de = nc.dram_tensor("l3de", [P, ncalls3 * 2 * NB], F32,
                          kind="ExternalInput")
    xmT = nc.dram_tensor("xmT", [P, W1 * P], BF16, kind="ExternalInput")
    xdT = nc.dram_tensor("xdT", [P, W2B * WW], BF16, kind="ExternalInput")
    wts = {}
    for nm, shape in [("W_rel1", [D, H]), ("W_root1", [D, H]),
                      ("W_rel2", [D, H]), ("W_root2", [D, H]),
                      ("W_rel3", [H, H]), ("W_root3", [H, H]),
                      ("W_lin", [H, O])]:
        wts[nm] = nc.dram_tensor(nm, shape, BF16, kind="ExternalInput")
    b1row = nc.dram_tensor("b1row", [1, H], BF16, kind="ExternalInput")
    b2col = nc.dram_tensor("b2col", [H, 1], F32, kind="ExternalInput")
    b3col = nc.dram_tensor("b3col", [H, 1], F32, kind="ExternalInput")
    blcol = nc.dram_tensor("blcol", [O, 1], F32, kind="ExternalInput")
    iota_in = nc.dram_tensor("iota", [P, WW], BF16, kind="ExternalInput")
    iotan_in = nc.dram_tensor("iotan", [P, WW], BF16, kind="ExternalInput")
    ones1 = nc.dram_tensor("ones1", [1, P], BF16, kind="ExternalInput")
    out = nc.dram_tensor("out", [O, RU], F32, kind="ExternalOutput")

    Relu = mybir.ActivationFunctionType.Relu
    Square = mybir.ActivationFunctionType.Square
    Ident = mybir.ActivationFunctionType.Identity
    Sigmoid = mybir.ActivationFunctionType.Sigmoid

    SW = WPG * WW              # cols per RS group per stage (512)

    with tile.TileContext(nc) as tc:
        with (
            tc.tile_pool(name="sbuf", bufs=4) as sb,
            tc.tile_pool(name="msgp", bufs=12) as msgp,
            tc.tile_pool(name="stgp", bufs=3) as stgp,
            tc.tile_pool(name="big", bufs=1) as bigp,
            tc.tile_pool(name="psum", bufs=3, space="PSUM") as ps,
            tc.tile_pool(name="psl", bufs=2, space="PSUM") as psl,
            tc.tile_pool(name="dram", bufs=1, space="DRAM") as dr,
        ):
            nc.gpsimd.load_library(library_config.mlp)

            # constants / weights to SBUF
            iota_f = bigp.tile([P, WW], BF16, tag="iota")
            nc.sync.dma_start(iota_f[:], iota_in[:])
            iota_n = bigp.tile([P, WW], BF16, tag="iotan")
            nc.sync.dma_start(iota_n[:], iotan_in[:])
            ones1_t = bigp.tile([1, P], BF16, tag="ones1")
            nc.sync.dma_start(ones1_t[:], ones1[:])
            wt = {}
            for nm in wts:
                shp = [D, H] if nm != "W_lin" else [H, O]
                wt[nm] = bigp.tile(shp, BF16, tag=nm, name=nm + "_t")
                nc.sync.dma_start(wt[nm][:], wts[nm][:])
            b1row_t = bigp.tile([1, H], BF16, tag="b1row")
            nc.sync.dma_start(b1row_t[:], b1row[:])
            bcol_t = {}
            for nm, t in [("b2", b2col), ("b3", b3col), ("bl", blcol)]:
                bcol_t[nm] = bigp.tile([t.shape[0], 1], F32, tag="bc_" + nm,
                                       name="bc_" + nm)
                nc.sync.dma_start(bcol_t[nm][:], t[:])

            xmT_t = bigp.tile([P, W1 * P], BF16, tag="xmT")
            nc.sync.dma_start(xmT_t[:], xmT[:])
            xdT_t = bigp.tile([P, W2B * WW], BF16, tag="xdT")
            nc.sync.dma_start(xdT_t[:], xdT[:])

            agg1 = bigp.tile([P, W1 * P], BF16, tag="agg1")
            agg2 = bigp.tile([P, W2B * WW], BF16, tag="agg2")
            agg3 = bigp.tile([P, W2B * WW], BF16, tag="agg3")
            user2 = bigp.tile([P, W2B * WW], BF16, tag="user2")
            user3 = bigp.tile([P, W2B * WW], BF16, tag="user3")
            outT = bigp.tile([O, W2B * WW], F32, tag="outT")

            mt = dr.tile([W1 * P, D], BF16, tag="movie")      # movie_x table
            cc_in = [dr.tile([NC, P, 2 * SW], BF16, tag=f"cci{g}",
                             name=f"cci{g}") for g in range(NWG)]
            cc_out = [dr.tile([P, 2 * SW], BF16, tag=f"cco{g}",
                              name=f"cco{g}") for g in range(NWG)]

            reg_cache = {}

            # ---------------- layer 1: segment sum into agg1 (feature-major)
            chunk1 = [0]

            def l1_next():
                k = chunk1[0]
                b, slot = divmod(k, SB1)
                if slot == 0:
                    nb = min(SB1, nch1 - b * SB1)
                    mtile = msgp.tile([P, SB1 * D], BF16, tag="m1", name="m1t",
                                      bufs=3)
                    nc.sync.dma_start(
                        mtile[:, :nb * D],
                        s1msg[:, b * SB1 * D: b * SB1 * D + nb * D])
                    dtile = msgp.tile([P, SB1], F32, tag="d1", name="d1t",
                                      bufs=3)
                    nc.sync.dma_start(dtile[:, :nb],
                                      s1dst[:, b * SB1: b * SB1 + nb])
                    l1_next.mtile, l1_next.dtile = mtile, dtile
                chunk1[0] += 1
                return (l1_next.mtile[:, slot * D:(slot + 1) * D],
                        l1_next.dtile[:, slot:slot + 1])

            for w in range(W1):
                n = int(sched1[w])
                dstc = agg1[:, w * P:(w + 1) * P]
                if n == 0:
                    nc.vector.memset(dstc, 0.0)
                    continue
                acc = ps.tile([P, P], F32, tag="win1", bufs=3)
                for j in range(n):
                    on_act = (chunk1[0] % 4 == 3)
                    msg_ap, dcol = l1_next()
                    S = sb.tile([P, P], BF16, tag="S1", bufs=8)
                    if on_act:
                        # exact for integer iota/dst: (i-d)^2 is 0 or >= 1
                        tmpS = sb.tile([P, P], BF16, tag="tmpS", name="tmpS",
                                       bufs=4)
                        nc.scalar.activation(tmpS[:], iota_f[:, :P], Square,
                                             bias=dcol)
                        nc.scalar.activation(S[:], tmpS[:], Relu, bias=1.0,
                                             scale=-1.0)
                    else:
                        nc.vector.tensor_scalar(
                            out=S[:], in0=iota_n[:, :P], scalar1=dcol,
                            scalar2=None, op0=mybir.AluOpType.is_equal)
                    nc.tensor.matmul(out=acc[:], lhsT=msg_ap, rhs=S[:],
                                     start=(j == 0), stop=(j == n - 1))
                nc.scalar.activation(dstc, acc[:], Ident)

            # ---------------- layer 1 linear -> movie table (node-major) ----
            for t in range(W1):
                pt = ps.tile([P, H], F32, tag="win1", bufs=3)
                sl = slice(t * P, (t + 1) * P)
                nc.tensor.matmul(out=pt[:], lhsT=ones1_t[:1, :],
                                 rhs=b1row_t[:1, :], start=True, stop=False)
                nc.tensor.matmul(out=pt[:], lhsT=agg1[:, sl],
                                 rhs=wt["W_rel1"][:], start=False, stop=False)
                nc.tensor.matmul(out=pt[:], lhsT=xmT_t[:, sl],
                                 rhs=wt["W_root1"][:], start=False, stop=True)
                mtt = sb.tile([P, H], BF16, tag="mv")
                nc.scalar.activation(mtt[:], pt[:], Relu)
                nc.scalar.dma_start(mt[t * P:(t + 1) * P, :], mtt[:])

            # ------- merged layers 2+3: shared S, two psums, chunked RS -----
            chunk3 = [0]

            SB3 = 2 * NB              # s2msg stream chunks per DMA batch
            IB3 = 4                   # gather calls per idx DMA batch

            def l23_next():
                k = chunk3[0]
                b, slot = divmod(k, NB)
                if slot == 0:
                    nb = min(NB, nch3 - b * NB)
                    if b % IB3 == 0:
                        ncol = min(IB3 * NB * 8, (ncalls3 - b) * NB * 8)
                        it = sb.tile([P, IB3 * NB * 8], I16, tag="idx",
                                     bufs=3)
                        nc.sync.dma_start(
                            it[:, :ncol],
                            l3idx[:, b * NB * 8: b * NB * 8 + ncol])
                        l23_next.it = it
                    itv = l23_next.it[:, (b % IB3) * NB * 8:
                                      (b % IB3) * NB * 8 + nb * 8]
                    gt = msgp.tile([P, NB, D], BF16, tag="mg", name="mgt",
                                   bufs=16)
                    v = nb * P
                    if v not in reg_cache:
                        reg_cache[v] = nc.gpsimd.to_reg(v)
                    q = reg_cache.setdefault("_q", [0])
                    nc.gpsimd.dma_gather(gt[:, :nb, :], mt.opt(),
                                         itv, v, reg_cache[v], D,
                                         queue_num=q[0] % 4)
                    q[0] += 1
                    if b % 2 == 0:
                        nbs = min(SB3, nch3 - b * NB)
                        st = msgp.tile([P, SB3 * D], BF16, tag="m2",
                                       name="m2t", bufs=8)
                        nc.scalar.dma_start(
                            st[:, :nbs * D],
                            s2msg[:, b * NB * D: b * NB * D + nbs * D])
                        l23_next.st = st
                    l23_next.soff = (b % 2) * NB * D
                    de = msgp.tile([P, 2 * NB], F32, tag="d3", name="d3t",
                                   bufs=8)
                    nc.sync.dma_start(de[:, :NB + nb],
                                      l3de[:, b * 2 * NB: b * 2 * NB + NB + nb])
                    ews = msgp.tile([P, NB], F32, tag="ews", name="ewst",
                                    bufs=8)
                    nc.scalar.activation(ews[:, :nb], de[:, NB:NB + nb],
                                         Sigmoid)
                    l23_next.gt = gt
                    l23_next.de, l23_next.ews = de, ews
                chunk3[0] += 1
                return (l23_next.gt[:, slot, :],
                        l23_next.st[:, l23_next.soff + slot * D:
                                    l23_next.soff + (slot + 1) * D],
                        l23_next.de[:, slot:slot + 1],
                        l23_next.ews[:, slot:slot + 1])

            for wg in range(NWG):
                for blk in range(NC):
                    if blk == 1 and wg >= 1:
                        # emit the previous group's RS one group late so its
                        # wait is satisfied on dequeue (no Pool-queue stall)
                        nc.gpsimd.collective_compute(
                            "ReduceScatter", mybir.AluOpType.add,
                            replica_groups=[list(range(NC))],
                            ins=[cc_in[wg - 1].opt()],
                            outs=[cc_out[wg - 1].opt()])
                    stg = stgp.tile([P, 2 * SW], BF16, tag="stg")
                    for w2 in range(WPG):
                        w = wg * (NC * WPG) + blk * WPG + w2
                        n = int(sched3[w])
                        c2 = stg[:, w2 * WW:(w2 + 1) * WW]
                        c3 = stg[:, SW + w2 * WW: SW + (w2 + 1) * WW]
                        if n == 0:
                            nc.vector.memset(c2, 0.0)
                            nc.vector.memset(c3, 0.0)
                            continue
                        accp = ps.tile([P, 2 * WW], F32, tag="win",
                                       name="accp", bufs=3)
                        acc2 = accp[:, 0:WW]
                        acc3 = accp[:, WW:2 * WW]
                        for j in range(n):
                            mv_ap, st_ap, dcol, ecol = l23_next()
                            S = sb.tile([P, WW], BF16, tag="S3", bufs=8)
                            nc.vector.tensor_scalar(
                                out=S[:], in0=iota_n[:], scalar1=dcol,
                                scalar2=ecol, op0=mybir.AluOpType.is_equal,
                                op1=mybir.AluOpType.mult)
                            nc.tensor.matmul(out=acc2, lhsT=st_ap, rhs=S[:],
                                             start=(j == 0), stop=(j == n - 1))
                            # start=False even at j==0: acc2's start already
                            # cleared the whole bank; has_written=0 here makes
                            # this first matmul overwrite, later ones add.
                            nc.tensor.matmul(out=acc3, lhsT=mv_ap, rhs=S[:],
                                             start=False, stop=(j == n - 1))
                        nc.scalar.activation(c2, acc2, Ident)
                        nc.scalar.activation(c3, acc3, Ident)
                    nc.scalar.dma_start(cc_in[wg][blk], stg[:])
            nc.gpsimd.collective_compute(
                "ReduceScatter", mybir.AluOpType.add,
                replica_groups=[list(range(NC))],
                ins=[cc_in[NWG - 1].opt()], outs=[cc_out[NWG - 1].opt()])

            # per-group linears: only the last group's chain is tail latency
            for wg in range(NWG):
                sl = slice(wg * SW, (wg + 1) * SW)
                nc.sync.dma_start(agg2[:, sl], cc_out[wg][:, 0:SW])
                nc.sync.dma_start(agg3[:, sl], cc_out[wg][:, SW:2 * SW])
                pt = psl.tile([P, 512], F32, tag="lin")
                nc.tensor.matmul(out=pt[:], lhsT=wt["W_rel2"][:],
                                 rhs=agg2[:, sl], start=True, stop=False)
                nc.tensor.matmul(out=pt[:], lhsT=wt["W_root2"][:],
                                 rhs=xdT_t[:, sl], start=False, stop=True)
                nc.scalar.activation(user2[:, sl], pt[:], Relu,
                                     bias=bcol_t["b2"][:])
                pt = psl.tile([P, 512], F32, tag="lin")
                nc.tensor.matmul(out=pt[:], lhsT=wt["W_rel3"][:],
                                 rhs=agg3[:, sl], start=True, stop=False)
                nc.tensor.matmul(out=pt[:], lhsT=wt["W_root3"][:],
                                 rhs=user2[:, sl], start=False, stop=True)
                nc.scalar.activation(user3[:, sl], pt[:], Relu,
                                     bias=bcol_t["b3"][:])
                pt = psl.tile([P, 512], F32, tag="lin")
                nc.tensor.matmul(out=pt[:O, :], lhsT=wt["W_lin"][:],
                                 rhs=user3[:, sl], start=True, stop=True)
                nc.scalar.activation(outT[:, sl], pt[:O, :], Ident,
                                     bias=bcol_t["bl"][:])
                lo = wg * SW
                hi = min((wg + 1) * SW, RU)
                nc.sync.dma_start(out[:, lo:hi], outT[:, lo:hi])

    lower_extended_insts(nc)
    return nc


# ----------------------------------------------------------------------- kernel
def prepare(x_meas, x_dem, src_m, dst_m, src_b, dst_b, edge_weight,
            W_rel1, b_rel1, W_root1, W_rel2, b_rel2, W_root2,
            W_rel3, b_rel3, W_root3, W_lin, b_lin):
    _install_birfix()

    x_meas = np.asarray(x_meas, np.float32)
    x_dem = np.asarray(x_dem, np.float32)
    src_m = np.asarray(src_m, np.int64)
    dst_m = np.asarray(dst_m, np.int64)
    src_b = np.asarray(src_b, np.int64)
    dst_b = np.asarray(dst_b, np.int64)
    ew = np.asarray(edge_weight, np.float32)
    x16 = x_meas.astype(BF16NP)

    # layer 1: dst-sharded
    core1 = dst_m // RM
    dloc1 = dst_m % RM
    win1 = dloc1 // P
    dstloc1 = (dloc1 % P).astype(np.float32)
    sched1, pos1 = _schedule(core1, win1, W1)
    nslots1 = int(sched1.sum()) * P

    # layers 2+3: src-sharded, windows ordered (group, block, sub-window)
    core3 = src_b // RM
    srcloc3 = src_b % RM
    blk3 = dst_b // RU
    uloc3 = dst_b % RU
    wib3 = uloc3 // WW
    dstloc3 = (uloc3 % WW).astype(np.float32)
    win3 = (wib3 // WPG) * (NC * WPG) + blk3 * WPG + (wib3 % WPG)
    sched3, pos3 = _schedule(core3, win3, WU)
    nslots3 = int(sched3.sum()) * P

    nc_prog = _build_program(sched1, sched3)

    iota = np.tile(np.arange(WW, dtype=np.float32), (P, 1)).astype(BF16NP)
    iotan = (-np.tile(np.arange(WW, dtype=np.float32), (P, 1))).astype(BF16NP)
    ones1 = np.ones((1, P), BF16NP)

    def padT(x, cols):
        o = np.zeros((P, cols), BF16NP)
        o[:, :x.shape[0]] = x.T.astype(BF16NP)
        return o

    in_maps = []
    for c in range(NC):
        m1 = core1 == c
        m3 = core3 == c
        m = {
            "s1msg": _pack_msg_stream(pos1[m1], x16[src_m[m1]], nslots1),
            "s1dst": _pack_col(pos1[m1], -dstloc1[m1], nslots1, 1.0),
            "s2msg": _pack_msg_stream(pos3[m3], x16[src_b[m3]], nslots3),
            "l3idx": _pack_idx_dma(pos3[m3], srcloc3[m3], nslots3),
            "l3de": _pack_dstew(pos3[m3], dstloc3[m3], ew[m3], nslots3),
            "xmT": padT(x_meas[c * RM:(c + 1) * RM], W1 * P),
            "xdT": padT(x_dem[c * RU:(c + 1) * RU], W2B * WW),
            "b1row": np.asarray(b_rel1, BF16NP).reshape(1, H),
            "b2col": np.asarray(b_rel2, np.float32).reshape(H, 1),
            "b3col": np.asarray(b_rel3, np.float32).reshape(H, 1),
            "blcol": np.asarray(b_lin, np.float32).reshape(O, 1),
            "iota": iota,
            "iotan": iotan,
            "ones1": ones1,
        }
        for nm, w in [("W_rel1", W_rel1), ("W_root1", W_root1),
                      ("W_rel2", W_rel2), ("W_root2", W_root2),
                      ("W_rel3", W_rel3), ("W_root3", W_root3),
                      ("W_lin", W_lin)]:
            m[nm] = np.asarray(w, BF16NP)
        in_maps.append(m)

    return nc_prog, in_maps


def kernel(**inputs):
    nc_prog, in_maps = prepare(**inputs)
    res = run_bass_kernel_spmd(nc_prog, in_maps, core_ids=list(range(NC)))
    outs = [res.results[c]["out"] for c in range(NC)]     # each [O, RU]
    full = np.concatenate(outs, axis=1).T                 # [N_D, O]
    return np.ascontiguousarray(full, dtype=np.float32)


# revision 13
# speedup vs baseline: 1.1875x; 1.1875x over previous
"""Trainium2 Bass kernel for the 3-layer weighted GraphConv encoder (v3).

Strategy (8 NeuronCores, SPMD single NEFF), all-bf16 datapath:
- Layer 1 (movie->movie): edges sharded by DST range. Messages x_meas[src_m]
  are PRE-GATHERED ON HOST into a contiguous bf16 stream (the indices are
  static, so the per-row SWDGE descriptor toll -- ~5-8ns/row of serial Q7
  time -- is avoided entirely). Segment-sum on the tensor engine via
  S[e,s] = (dstloc[e]==s) selection matrices; feature-major agg kept in SBUF.
- Layers 2+3 (movie->user, shared edge set): edges sharded by SRC range so
  the layer-3 gather of movie_x (device-computed) reads only the core-local
  [RM, D] bf16 table via dma_gather (256B rows). Layer-2 messages
  x_meas[src_b] come from a host-pregathered bf16 stream. One shared
  S = (dstloc==s)*sigmoid(ew) per 128-edge chunk (256-dst windows) feeds two
  matmuls (acc2 from the host stream, acc3 from the gathered movie rows).
- Partial [128, N_D]-feature-major aggregates are reduced across cores with
  5 chunked bf16 ReduceScatters. Windows are processed in (group, block,
  sub-window) order so each RS piece completes early; each RS is emitted one
  group late on the Pool queue so its semaphore wait never stalls the
  gather stream. The small dense linears run replicated per-core at the end.

The per-window chunk schedule is data-dependent; it is computed from the
actual inputs at kernel() time (max over cores per window) and baked into
the program, with per-core padding to the shared schedule (padded slots
have msg rows = 0 and dstloc = -1 so they contribute nothing).
"""

import math

import ml_dtypes
import numpy as np
import orjson

import concourse.bass as bass
import concourse.mybir as mybir
import concourse.tile as tile
from concourse import library_config
from concourse.library_overlay import lower_extended_insts
from concourse.bass_utils import run_bass_kernel_spmd

BF16NP = ml_dtypes.bfloat16

# ---------------------------------------------------------------- BIR legalize
# The pinned walrus build accepts at most one sync-wait and one sync-update
# per instruction; Tile emits several. Hoist extras onto EventSemaphore nops.
_DMA_OPCODES = {
    "DMACopy", "TensorLoad", "TensorSave", "ISA", "CollectiveCompute",
    "DmaTranspose", "TriggerDma",
}
_lg_counter = [0]


def _lg_nop(inst, wait=None, update=None):
    _lg_counter[0] += 1
    return {
        "name": f"lg{_lg_counter[0]}",
        "opcode": "EventSemaphore",
        "engine": inst["engine"],
        "ins": [],
        "outs": [],
        "debug": inst.get("debug", 0),
        "sync_info": {
            "on_wait": [wait] if wait else [],
            "on_update": [update] if update else [],
        },
    }


def _lg_walk(block, stats):
    out = []
    for inst in block.get("instructions", []):
        si = inst.get("sync_info")
        trailing = []
        if si:
            ows = si.get("on_wait") or []
            if len(ows) > 1:
                stats[0] += len(ows) - 1
                for w in ows[:-1]:
                    out.append(_lg_nop(inst, wait=w))
                si["on_wait"] = [ows[-1]]
            ous = si.get("on_update") or []
            if len(ous) > 1 and inst.get("opcode") not in _DMA_OPCODES:
                stats[1] += len(ous) - 1
                for u in ous[1:]:
                    trailing.append(_lg_nop(inst, update=u))
                si["on_update"] = [ous[0]]
        out.append(inst)
        out.extend(trailing)
    block["instructions"] = out
    for sb in block.get("blocks") or []:
        _lg_walk(sb, stats)


def legalize_bir_json(bir_json: bytes) -> bytes:
    d = orjson.loads(bir_json)
    stats = [0, 0]
    for fn in d.get("functions", []):
        for b in fn.get("blocks", []):
            _lg_walk(b, stats)
    return orjson.dumps(d)


def _install_birfix():
    import concourse.bass_utils as bu
    import concourse.bass2jax as b2j

    if getattr(bu, "_birfix_installed", False):
        return
    orig = bu.compile_bir_kernel

    def wrapper(bir_json, tmpdir, neff_name="file.neff"):
        return orig(legalize_bir_json(bir_json), tmpdir, neff_name)

    bu.compile_bir_kernel = wrapper
    bu._birfix_installed = True
    b2j.compile_bir_kernel = wrapper


# ------------------------------------------------------------------- constants
N_M, N_D, E, D, H, O = 50000, 20000, 600000, 128, 128, 64
NC = 8
P = 128
WW = 128                  # layer-2/3 dst window width
RM = N_M // NC            # 6250 movie rows per core
RU = N_D // NC            # 2500 user rows per core
W1 = math.ceil(RM / P)    # 49 windows for layer 1
W2B = math.ceil(RU / WW)  # 10 user windows per dst block
NWG = 5                   # ReduceScatter groups (2 sub-windows each)
WPG = W2B // NWG          # sub-windows per group (2)
WU = NC * W2B             # 80 user windows for layers 2+3
NB = 8                    # chunks per dma_gather batch (>1024 idx/call faults)
SB1 = 16                  # layer-1 stream chunks per DMA batch
F32 = mybir.dt.float32
BF16 = mybir.dt.bfloat16
I16 = mybir.dt.int16


# ---------------------------------------------------------------- host-side prep
def _schedule(core, win, nwin):
    """Shared chunk schedule + per-edge slot positions.

    Returns (sched [nwin] = chunks per window, pos [E'] = slot index of each
    edge within its core's stream). All cores share sched; each core's stream
    is sched.sum()*P slots with window w's run at off[w]."""
    counts = np.zeros((NC, nwin), np.int64)
    np.add.at(counts, (core, win), 1)
    sched = (counts.max(axis=0) + P - 1) // P
    run_len = sched * P
    off = np.concatenate(([0], np.cumsum(run_len)[:-1]))

    # rank of each edge within its (core, win) bucket
    order = np.lexsort((win, core))
    inv = np.empty_like(order)
    inv[order] = np.arange(len(order))
    flat = core * nwin + win
    sort_flat = flat[order]
    starts = np.concatenate(([0], np.nonzero(np.diff(sort_flat))[0] + 1))
    run_start = np.zeros(len(order), np.int64)
    run_start[starts] = starts
    run_start = np.maximum.accumulate(run_start)
    rank = (np.arange(len(order)) - run_start)[inv]

    pos = off[win] + rank
    return sched, pos


def _pack_msg_stream(pos_c, rows_bf16, nslots):
    """Scatter pregathered bf16 rows [n, D] into the DMA stream layout
    [P, nchunks*D]: slot s -> (chunk s//P, partition s%P)."""
    arr = np.zeros((nslots, D), BF16NP)
    arr[pos_c] = rows_bf16
    nch = nslots // P
    return np.ascontiguousarray(
        arr.reshape(nch, P, D).transpose(1, 0, 2).reshape(P, nch * D))


def _pack_col(pos_c, vals, nslots, fill):
    arr = np.full(nslots, fill, np.float32)
    arr[pos_c] = vals
    return np.ascontiguousarray(arr.reshape(-1, P).T)


def _pack_dstew(pos_c, dst_vals, ew_vals, nslots):
    """Per dma_gather call k: cols [k*2NB, k*2NB+NB) = negated dstloc chunks,
    cols [k*2NB+NB, k*2NB+2NB) = raw edge weights."""
    dc = _pack_col(pos_c, -dst_vals, nslots, 1.0)      # [P, nch]
    ec = _pack_col(pos_c, ew_vals, nslots, 0.0)
    nch = nslots // P
    ncalls = math.ceil(nch / NB)
    out = np.zeros((P, ncalls * 2 * NB), np.float32)
    for k in range(ncalls):
        nb = min(NB, nch - k * NB)
        out[:, k * 2 * NB: k * 2 * NB + nb] = dc[:, k * NB: k * NB + nb]
        out[:, k * 2 * NB + NB: k * 2 * NB + NB + nb] = ec[:, k * NB: k * NB + nb]
    return out


def _pack_idx_dma(pos_c, idx_vals, nslots):
    """idx stream -> DMA layout [P, ncalls*NB*8]: per dma_gather call k
    (NB chunks), index j -> partition j%16 (replicated x8), col k*NB*8+j//16."""
    idx_a = np.zeros(nslots, np.int16)
    idx_a[pos_c] = idx_vals.astype(np.int16)
    nchunks = nslots // P
    ncalls = math.ceil(nchunks / NB)
    out = np.zeros((P, ncalls * NB * 8), np.int16)
    for k in range(ncalls):
        nb = min(NB, nchunks - k * NB)
        call = idx_a[k * NB * P: k * NB * P + nb * P]
        blk = call.reshape(nb * 8, 16).T               # [16, nb*8]
        out[:, k * NB * 8: k * NB * 8 + nb * 8] = np.tile(blk, (8, 1))
    return out


# --------------------------------------------------------------- device program
def _build_program(sched1, sched3):
    nc = bass.Bass(trn_type="TRN2", num_devices=NC, num_swdge_queues=4)

    nch1 = int(sched1.sum())
    nch3 = int(sched3.sum())
    ncalls3 = math.ceil(nch3 / NB)

    # ---- kernel I/O ----
    s1msg = nc.dram_tensor("s1msg", [P, nch1 * D], BF16, kind="ExternalInput")
    s1dst = nc.dram_tensor("s1dst", [P, nch1], F32, kind="ExternalInput")
    s2msg = nc.dram_tensor("s2msg", [P, nch3 * D], BF16, kind="ExternalInput")
    l3idx = nc.dram_tensor("l3idx", [P, ncalls3 * NB * 8], I16,
                           kind="ExternalInput")
    l3de = nc.dram_tensor("l3de", [P, ncalls3 * 2 * NB], F32,
                          kind="ExternalInput")
    xmT = nc.dram_tensor("xmT", [P, W1 * P], BF16, kind="ExternalInput")
    xdT = nc.dram_tensor("xdT", [P, W2B * WW], BF16, kind="ExternalInput")
    wts = {}
    for nm, shape in [("W_rel1", [D, H]), ("W_root1", [D, H]),
                      ("W_rel2", [D, H]), ("W_root2", [D, H]),
                      ("W_rel3", [H, H]), ("W_root3", [H, H]),
                      ("W_lin", [H, O])]:
        wts[nm] = nc.dram_tensor(nm, shape, BF16, kind="ExternalInput")
    b1row = nc.dram_tensor("b1row", [1, H], BF16, kind="ExternalInput")
    b2col = nc.dram_tensor("b2col", [H, 1], F32, kind="ExternalInput")
    b3col = nc.dram_tensor("b3col", [H, 1], F32, kind="ExternalInput")
    blcol = nc.dram_tensor("blcol", [O, 1], F32, kind="ExternalInput")
    iota_in = nc.dram_tensor("iota", [P, WW], BF16, kind="ExternalInput")
    iotan_in = nc.dram_tensor("iotan", [P, WW], BF16, kind="ExternalInput")
    ones1 = nc.dram_tensor("ones1", [1, P], BF16, kind="ExternalInput")
    out = nc.dram_tensor("out", [O, RU], F32, kind="ExternalOutput")

    Relu = mybir.ActivationFunctionType.Relu
    Square = mybir.ActivationFunctionType.Square
    Ident = mybir.ActivationFunctionType.Identity
    Sigmoid = mybir.ActivationFunctionType.Sigmoid

    SW = WPG * WW              # cols per RS group per stage (512)

    with tile.TileContext(nc) as tc:
        with (
            tc.tile_pool(name="sbuf", bufs=4) as sb,
            tc.tile_pool(name="msgp", bufs=12) as msgp,
            tc.tile_pool(name="stgp", bufs=3) as stgp,
            tc.tile_pool(name="big", bufs=1) as bigp,
            tc.tile_pool(name="psum", bufs=3, space="PSUM") as ps,
            tc.tile_pool(name="psl", bufs=2, space="PSUM") as psl,
            tc.tile_pool(name="dram", bufs=1, space="DRAM") as dr,
        ):
            nc.gpsimd.load_library(library_config.mlp)

            # constants / weights to SBUF
            iota_f = bigp.tile([P, WW], BF16, tag="iota")
            nc.sync.dma_start(iota_f[:], iota_in[:])
            iota_n = bigp.tile([P, WW], BF16, tag="iotan")
            nc.sync.dma_start(iota_n[:], iotan_in[:])
            ones1_t = bigp.tile([1, P], BF16, tag="ones1")
            nc.sync.dma_start(ones1_t[:], ones1[:])
            wt = {}
            for nm in wts:
                shp = [D, H] if nm != "W_lin" else [H, O]
                wt[nm] = bigp.tile(shp, BF16, tag=nm, name=nm + "_t")
                nc.sync.dma_start(wt[nm][:], wts[nm][:])
            b1row_t = bigp.tile([1, H], BF16, tag="b1row")
            nc.sync.dma_start(b1row_t[:], b1row[:])
            bcol_t = {}
            for nm, t in [("b2", b2col), ("b3", b3col), ("bl", blcol)]:
                bcol_t[nm] = bigp.tile([t.shape[0], 1], F32, tag="bc_" + nm,
                                       name="bc_" + nm)
                nc.sync.dma_start(bcol_t[nm][:], t[:])

            xmT_t = bigp.tile([P, W1 * P], BF16, tag="xmT")
            nc.sync.dma_start(xmT_t[:], xmT[:])
            xdT_t = bigp.tile([P, W2B * WW], BF16, tag="xdT")
            nc.sync.dma_start(xdT_t[:], xdT[:])

            agg1 = bigp.tile([P, W1 * P], BF16, tag="agg1")
            agg2 = bigp.tile([P, W2B * WW], BF16, tag="agg2")
            agg3 = bigp.tile([P, W2B * WW], BF16, tag="agg3")
            user2 = bigp.tile([P, W2B * WW], BF16, tag="user2")
            user3 = bigp.tile([P, W2B * WW], BF16, tag="user3")
            outT = bigp.tile([O, W2B * WW], F32, tag="outT")

            mt = dr.tile([W1 * P, D], BF16, tag="movie")      # movie_x table
            cc_in = [dr.tile([NC, P, 2 * SW], BF16, tag=f"cci{g}",
                             name=f"cci{g}") for g in range(NWG)]
            cc_out = [dr.tile([P, 2 * SW], BF16, tag=f"cco{g}",
                              name=f"cco{g}") for g in range(NWG)]

            reg_cache = {}

            # ---------------- layer 1: segment sum into agg1 (feature-major)
            chunk1 = [0]

            def l1_next():
                k = chunk1[0]
                b, slot = divmod(k, SB1)
                if slot == 0:
                    nb = min(SB1, nch1 - b * SB1)
                    mtile = msgp.tile([P, SB1 * D], BF16, tag="m1", name="m1t",
                                      bufs=3)
                    nc.sync.dma_start(
                        mtile[:, :nb * D],
                        s1msg[:, b * SB1 * D: b * SB1 * D + nb * D])
                    dtile = msgp.tile([P, SB1], F32, tag="d1", name="d1t",
                                      bufs=3)
                    nc.sync.dma_start(dtile[:, :nb],
                                      s1dst[:, b * SB1: b * SB1 + nb])
                    l1_next.mtile, l1_next.dtile = mtile, dtile
                chunk1[0] += 1
                return (l1_next.mtile[:, slot * D:(slot + 1) * D],
                        l1_next.dtile[:, slot:slot + 1])

            for w in range(W1):
                n = int(sched1[w])
                dstc = agg1[:, w * P:(w + 1) * P]
                if n == 0:
                    nc.vector.memset(dstc, 0.0)
                    continue
                acc = ps.tile([P, P], F32, tag="win1", bufs=3)
                for j in range(n):
                    msg_ap, dcol = l1_next()
                    S = sb.tile([P, P], BF16, tag="S1", bufs=8)
                    nc.vector.tensor_scalar(
                        out=S[:], in0=iota_n[:, :P], scalar1=dcol,
                        scalar2=None, op0=mybir.AluOpType.is_equal)
                    nc.tensor.matmul(out=acc[:], lhsT=msg_ap, rhs=S[:],
                                     start=(j == 0), stop=(j == n - 1))
                nc.scalar.activation(dstc, acc[:], Ident)

            # ---------------- layer 1 linear -> movie table (node-major) ----
            for t in range(W1):
                pt = ps.tile([P, H], F32, tag="win1", bufs=3)
                sl = slice(t * P, (t + 1) * P)
                nc.tensor.matmul(out=pt[:], lhsT=ones1_t[:1, :],
                                 rhs=b1row_t[:1, :], start=True, stop=False)
                nc.tensor.matmul(out=pt[:], lhsT=agg1[:, sl],
                                 rhs=wt["W_rel1"][:], start=False, stop=False)
                nc.tensor.matmul(out=pt[:], lhsT=xmT_t[:, sl],
                                 rhs=wt["W_root1"][:], start=False, stop=True)
                mtt = sb.tile([P, H], BF16, tag="mv")
                nc.scalar.activation(mtt[:], pt[:], Relu)
                nc.scalar.dma_start(mt[t * P:(t + 1) * P, :], mtt[:])

            # ------- merged layers 2+3: shared S, two psums, chunked RS -----
            chunk3 = [0]

            SB3 = 2 * NB              # s2msg stream chunks per DMA batch
            IB3 = 4                   # gather calls per idx DMA batch

            def l23_next():
                k = chunk3[0]
                b, slot = divmod(k, NB)
                if slot == 0:
                    nb = min(NB, nch3 - b * NB)
                    if b % IB3 == 0:
                        ncol = min(IB3 * NB * 8, (ncalls3 - b) * NB * 8)
                        it = sb.tile([P, IB3 * NB * 8], I16, tag="idx",
                                     bufs=3)
                        nc.sync.dma_start(
                            it[:, :ncol],
                            l3idx[:, b * NB * 8: b * NB * 8 + ncol])
                        l23_next.it = it
                    itv = l23_next.it[:, (b % IB3) * NB * 8:
                                      (b % IB3) * NB * 8 + nb * 8]
                    gt = msgp.tile([P, NB, D], BF16, tag="mg", name="mgt",
                                   bufs=16)
                    v = nb * P
                    if v not in reg_cache:
                        reg_cache[v] = nc.gpsimd.to_reg(v)
                    q = reg_cache.setdefault("_q", [0])
                    nc.gpsimd.dma_gather(gt[:, :nb, :], mt.opt(),
                                         itv, v, reg_cache[v], D,
                                         queue_num=q[0] % 4)
                    q[0] += 1
                    if b % 2 == 0:
                        nbs = min(SB3, nch3 - b * NB)
                        st = msgp.tile([P, SB3 * D], BF16, tag="m2",
                                       name="m2t", bufs=8)
                        nc.sync.dma_start(
                            st[:, :nbs * D],
                            s2msg[:, b * NB * D: b * NB * D + nbs * D])
                        l23_next.st = st
                    l23_next.soff = (b % 2) * NB * D
                    de = msgp.tile([P, 2 * NB], F32, tag="d3", name="d3t",
                                   bufs=8)
                    nc.sync.dma_start(de[:, :NB + nb],
                                      l3de[:, b * 2 * NB: b * 2 * NB + NB + nb])
                    ews = msgp.tile([P, NB], F32, tag="ews", name="ewst",
                                    bufs=8)
                    nc.scalar.activation(ews[:, :nb], de[:, NB:NB + nb],
                                         Sigmoid)
                    l23_next.gt = gt
                    l23_next.de, l23_next.ews = de, ews
                chunk3[0] += 1
                return (l23_next.gt[:, slot, :],
                        l23_next.st[:, l23_next.soff + slot * D:
                                    l23_next.soff + (slot + 1) * D],
                        l23_next.de[:, slot:slot + 1],
                        l23_next.ews[:, slot:slot + 1])

            for wg in range(NWG):
                for blk in range(NC):
                    if blk == 1 and wg >= 1:
                        # emit the previous group's RS one group late so its
                        # wait is satisfied on dequeue (no Pool-queue stall)
                        nc.gpsimd.collective_compute(
                            "ReduceScatter", mybir.AluOpType.add,
                            replica_groups=[list(range(NC))],
                            ins=[cc_in[wg - 1].opt()],
                            outs=[cc_out[wg - 1].opt()])
                    stg = stgp.tile([P, 2 * SW], BF16, tag="stg")
                    for w2 in range(WPG):
                        w = wg * (NC * WPG) + blk * WPG + w2
                        n = int(sched3[w])
                        c2 = stg[:, w2 * WW:(w2 + 1) * WW]
                        c3 = stg[:, SW + w2 * WW: SW + (w2 + 1) * WW]
                        if n == 0:
                            nc.vector.memset(c2, 0.0)
                            nc.vector.memset(c3, 0.0)
                            continue
                        accp = ps.tile([P, 2 * WW], F32, tag="win",
                                       name="accp", bufs=3)
                        acc2 = accp[:, 0:WW]
                        acc3 = accp[:, WW:2 * WW]
                        for j in range(n):
                            mv_ap, st_ap, dcol, ecol = l23_next()
                            S = sb.tile([P, WW], BF16, tag="S3", bufs=8)
                            nc.vector.tensor_scalar(
                                out=S[:], in0=iota_n[:], scalar1=dcol,
                                scalar2=ecol, op0=mybir.AluOpType.is_equal,
                                op1=mybir.AluOpType.mult)
                            nc.tensor.matmul(out=acc2, lhsT=st_ap, rhs=S[:],
                                             start=(j == 0), stop=(j == n - 1))
                            # start=False even at j==0: acc2's start already
                            # cleared the whole bank; has_written=0 here makes
                            # this first matmul overwrite, later ones add.
                            nc.tensor.matmul(out=acc3, lhsT=mv_ap, rhs=S[:],
                                             start=False, stop=(j == n - 1))
                        nc.scalar.activation(c2, acc2, Ident)
                        nc.scalar.activation(c3, acc3, Ident)
                    nc.scalar.dma_start(cc_in[wg][blk], stg[:])
            nc.gpsimd.collective_compute(
                "ReduceScatter", mybir.AluOpType.add,
                replica_groups=[list(range(NC))],
                ins=[cc_in[NWG - 1].opt()], outs=[cc_out[NWG - 1].opt()])

            # per-group linears: only the last group's chain is tail latency
            for wg in range(NWG):
                sl = slice(wg * SW, (wg + 1) * SW)
                nc.sync.dma_start(agg2[:, sl], cc_out[wg][:, 0:SW])
                nc.sync.dma_start(agg3[:, sl], cc_out[wg][:, SW:2 * SW])
                pt = psl.tile([P, 512], F32, tag="lin")
                nc.tensor.matmul(out=pt[:], lhsT=wt["W_rel2"][:],
                                 rhs=agg2[:, sl], start=True, stop=False)
                nc.tensor.matmul(out=pt[:], lhsT=wt["W_root2"][:],
                                 rhs=xdT_t[:, sl], start=False, stop=True)
                nc.scalar.activation(user2[:, sl], pt[:], Relu,
                                     bias=bcol_t["b2"][:])
                pt = psl.tile([P, 512], F32, tag="lin")
                nc.tensor.matmul(out=pt[:], lhsT=wt["W_rel3"][:],
                                 rhs=agg3[:, sl], start=True, stop=False)
                nc.tensor.matmul(out=pt[:], lhsT=wt["W_root3"][:],
                                 rhs=user2[:, sl], start=False, stop=True)
                nc.scalar.activation(user3[:, sl], pt[:], Relu,
                                     bias=bcol_t["b3"][:])
                pt = psl.tile([P, 512], F32, tag="lin")
                nc.tensor.matmul(out=pt[:O, :], lhsT=wt["W_lin"][:],
                                 rhs=user3[:, sl], start=True, stop=True)
                nc.scalar.activation(outT[:, sl], pt[:O, :], Ident,
                                     bias=bcol_t["bl"][:])
                lo = wg * SW
                hi = min((wg + 1) * SW, RU)
                nc.sync.dma_start(out[:, lo:hi], outT[:, lo:hi])

    lower_extended_insts(nc)
    return nc


# ----------------------------------------------------------------------- kernel
def prepare(x_meas, x_dem, src_m, dst_m, src_b, dst_b, edge_weight,
            W_rel1, b_rel1, W_root1, W_rel2, b_rel2, W_root2,
            W_rel3, b_rel3, W_root3, W_lin, b_lin):
    _install_birfix()

    x_meas = np.asarray(x_meas, np.float32)
    x_dem = np.asarray(x_dem, np.float32)
    src_m = np.asarray(src_m, np.int64)
    dst_m = np.asarray(dst_m, np.int64)
    src_b = np.asarray(src_b, np.int64)
    dst_b = np.asarray(dst_b, np.int64)
    ew = np.asarray(edge_weight, np.float32)
    x16 = x_meas.astype(BF16NP)

    # layer 1: dst-sharded
    core1 = dst_m // RM
    dloc1 = dst_m % RM
    win1 = dloc1 // P
    dstloc1 = (dloc1 % P).astype(np.float32)
    sched1, pos1 = _schedule(core1, win1, W1)
    nslots1 = int(sched1.sum()) * P

    # layers 2+3: src-sharded, windows ordered (group, block, sub-window)
    core3 = src_b // RM
    srcloc3 = src_b % RM
    blk3 = dst_b // RU
    uloc3 = dst_b % RU
    wib3 = uloc3 // WW
    dstloc3 = (uloc3 % WW).astype(np.float32)
    win3 = (wib3 // WPG) * (NC * WPG) + blk3 * WPG + (wib3 % WPG)
    sched3, pos3 = _schedule(core3, win3, WU)
    nslots3 = int(sched3.sum()) * P

    nc_prog = _build_program(sched1, sched3)

    iota = np.tile(np.arange(WW, dtype=np.float32), (P, 1)).astype(BF16NP)
    iotan = (-np.tile(np.arange(WW, dtype=np.float32), (P, 1))).astype(BF16NP)
    ones1 = np.ones((1, P), BF16NP)

    def padT(x, cols):
        o = np.zeros((P, cols), BF16NP)
        o[:, :x.shape[0]] = x.T.astype(BF16NP)
        return o

    in_maps = []
    for c in range(NC):
        m1 = core1 == c
        m3 = core3 == c
        m = {
            "s1msg": _pack_msg_stream(pos1[m1], x16[src_m[m1]], nslots1),
            "s1dst": _pack_col(pos1[m1], -dstloc1[m1], nslots1, 1.0),
            "s2msg": _pack_msg_stream(pos3[m3], x16[src_b[m3]], nslots3),
            "l3idx": _pack_idx_dma(pos3[m3], srcloc3[m3], nslots3),
            "l3de": _pack_dstew(pos3[m3], dstloc3[m3], ew[m3], nslots3),
            "xmT": padT(x_meas[c * RM:(c + 1) * RM], W1 * P),
            "xdT": padT(x_dem[c * RU:(c + 1) * RU], W2B * WW),
            "b1row": np.asarray(b_rel1, BF16NP).reshape(1, H),
            "b2col": np.asarray(b_rel2, np.float32).reshape(H, 1),
            "b3col": np.asarray(b_rel3, np.float32).reshape(H, 1),
            "blcol": np.asarray(b_lin, np.float32).reshape(O, 1),
            "iota": iota,
            "iotan": iotan,
            "ones1": ones1,
        }
        for nm, w in [("W_rel1", W_rel1), ("W_root1", W_root1),
                      ("W_rel2", W_rel2), ("W_root2", W_root2),
                      ("W_rel3", W_rel3), ("W_root3", W_root3),
                      ("W_lin", W_lin)]:
            m[nm] = np.asarray(w, BF16NP)
        in_maps.append(m)

    return nc_prog, in_maps


def kernel(**inputs):
    nc_prog, in_maps = prepare(**inputs)
    res = run_bass_kernel_spmd(nc_prog, in_maps, core_ids=list(range(NC)))
    outs = [res.results[c]["out"] for c in range(NC)]     # each [O, RU]
    full = np.concatenate(outs, axis=1).T                 # [N_D, O]
    return np.ascontiguousarray(full, dtype=np.float32)
